# revision 31
# baseline (speedup 1.0000x reference)
"""DiffuMamba3 Trainium2 kernel.

Sharding: 8 cores = 2 batch groups (batch b = core//4) x 4 tensor-parallel
ranks (r = core%4). Within a group: d_inner of each SSM, attention heads,
and MLP hidden are sharded 4 ways (one AllReduce per sublayer output);
the tied lm_head is vocab-sharded 4 ways within the group (no comm).

On-chip layout is "T-layout": features on SBUF partitions, time along the
free dimension. All matmuls consume activations directly (contraction dim
on partitions) and produce T-layout outputs with weights as lhsT. The
Mamba recurrence uses the DVE tensor_tensor_scan instruction (one
recurrence per partition along the free dim), vectorized over (d, s) by
iterating s and scanning [128 d x 512 t] tiles.

adaLN folding: LayerNorm runs on-device; the (1+scale) modulation is
folded into the consuming weights host-side (scale is per-batch, constant
per core) and the shift contribution becomes a per-output-feature bias
added during PSUM eviction. Attention's V-bias is applied to attn@V's
output (softmax rows sum to 1, so this is exact).
"""

import os
import numpy as np
import ml_dtypes

import concourse.bass as bass
import concourse.bacc as bacc
import concourse.mybir as mybir
import concourse.tile as tile
from concourse.bass_utils import run_bass_kernel_spmd

F32 = mybir.dt.float32
BF16 = mybir.dt.bfloat16
AF = mybir.ActivationFunctionType
ALU = mybir.AluOpType
AX = mybir.AxisListType
BF = ml_dtypes.bfloat16

# model dims (fixed by the problem)
V, D, L, COND = 50304, 512, 512, 128
DI, DS, DT = 1024, 16, 32
NH, HD = 8, 64
HID = 2 * D
N_LAYERS = 4
ATTN = (1, 3)
FREQ = 256

# sharding
TP = 4                 # ranks per batch group
DIr = DI // TP         # 256 -> 2 tiles of 128
HIDr = HID // TP       # 256 -> 2 tiles
NHr = NH // TP         # 2 heads/rank
VS = V // TP           # 12576 vocab columns per core
KD = D // 128          # 4 K-chunks over the model dim
JD = DIr // 128        # 2 d_inner tiles per rank
JH = HIDr // 128       # 2 hidden tiles per rank
NVC = (VS + 511) // 512  # 25 vocab chunks (24x512 + 288)

DEBUG_TAPS = bool(int(os.environ.get("KERNEL_DEBUG_TAPS", "0")))
N_BUILD_BLOCKS = int(os.environ.get("KERNEL_NBLOCKS", str(N_LAYERS)))

_cache = {}


def _silu(x):
    return x / (1.0 + np.exp(-x))


def _host_cond(t, p):
    half = FREQ // 2
    freqs = np.exp(-np.log(10000.0) * np.arange(half, dtype=np.float32) / half)
    args = t[:, None].astype(np.float32) * freqs[None, :]
    emb = np.concatenate([np.cos(args), np.sin(args)], axis=-1)
    return _silu(emb @ p['t_w1'] + p['t_b1']) @ p['t_w2'] + p['t_b2']


def _mod3(c, w, b):
    m = c @ w + b
    return np.split(m, 3, axis=-1)  # shift, scale, gate each (1, D)


def _bf(a):
    return np.ascontiguousarray(a.astype(BF))


def _vec_tiles(v, nt):
    # (nt*128,) fp32 -> (nt, 128, 1) for per-partition scalar DMA loads
    return np.ascontiguousarray(v.astype(np.float32).reshape(nt, 128, 1))


def prepare_inputs(input_ids, t, params):
    """Returns list of 8 in_maps (one per core)."""
    input_ids = np.asarray(input_ids)
    t = np.asarray(t, np.float32)
    p = {k: np.asarray(v, np.float32) if k != 'blocks' else v
         for k, v in params.items()}
    blocks = [{k: (np.asarray(v, np.float32) if not isinstance(v, dict) else
                   {k2: np.asarray(v2, np.float32) for k2, v2 in v.items()})
               for k, v in bp.items()} for bp in p['blocks']]

    c = _host_cond(t, p)  # (B, COND)
    tok = p['tok_emb']
    ident = np.eye(128, dtype=np.float32)

    # rank-only (batch-independent) tensors, computed once per rank
    wlm_r = [_bf(tok[r * VS:(r + 1) * VS].T) for r in range(TP)]

    in_maps = []
    for core in range(8):
        g, r = core // TP, core % TP
        selbc = np.zeros((2 * DS, 2 * DS * 128), np.float32)
        for s in range(DS):
            selbc[s, s * 128:(s + 1) * 128] = 1.0
            selbc[DS + s, (DS + s) * 128:(DS + s + 1) * 128] = 1.0
        m = {'ident': _bf(ident), 'selbc': _bf(selbc)}

        x0 = tok[input_ids[g]] + p['pos_emb'][:L]
        m['x0T'] = np.ascontiguousarray(x0.T.astype(np.float32))  # (D, L)

        cb = c[g:g + 1]  # (1, COND)

        for i, bp in enumerate(blocks):
            pre = f'b{i}_'
            if i in ATTN:
                sh, sc, ga = _mod3(cb, bp['mod_a_w'], bp['mod_a_b'])
                sh, sc, ga = sh[0], sc[0], ga[0]
                qkv = bp['qkv']
                Wq, Wk, Wv = qkv[:, :D], qkv[:, D:2 * D], qkv[:, 2 * D:]
                hs = slice(r * NHr * HD, (r + 1) * NHr * HD)
                m[pre + 'wq'] = _bf((Wq * (1 + sc)[:, None])[:, hs] / np.sqrt(HD))
                m[pre + 'bq'] = _vec_tiles((Wq[:, hs].T @ sh) / np.sqrt(HD), 1)
                m[pre + 'wk'] = _bf((Wk * (1 + sc)[:, None])[:, hs])
                m[pre + 'bk'] = _vec_tiles(Wk[:, hs].T @ sh, 1)
                m[pre + 'wv'] = _bf((Wv * (1 + sc)[:, None])[:, hs])
                m[pre + 'bv'] = _vec_tiles(Wv[:, hs].T @ sh, 1)
                m[pre + 'wo'] = _bf(bp['out_proj'][hs, :])
                m[pre + 'ga'] = _vec_tiles(ga, KD)
            else:
                sh, sc, ga = _mod3(cb, bp['mod_m_w'], bp['mod_m_b'])
                sh, sc, ga = sh[0], sc[0], ga[0]
                for d, sp in (('f', bp['ssm_f']), ('b', bp['ssm_b'])):
                    q = pre + d + '_'
                    rsl = slice(r * DIr, (r + 1) * DIr)
                    # permute x-columns so this rank's shard is first
                    perm = np.concatenate([np.arange(r * DIr, (r + 1) * DIr),
                                           np.arange(0, r * DIr),
                                           np.arange((r + 1) * DIr, DI)])
                    Wx = sp['in_proj'][:, :DI][:, perm]
                    Wz = sp['in_proj'][:, DI:][:, rsl]
                    m[q + 'wx'] = _bf(Wx * (1 + sc)[:, None])
                    m[q + 'bx'] = _vec_tiles(Wx.T @ sh, DI // 128)
                    m[q + 'wz'] = _bf(Wz * (1 + sc)[:, None])
                    m[q + 'bz'] = _vec_tiles(Wz.T @ sh, JD)
                    m[q + 'xp'] = _bf(sp['x_proj'][perm, :])  # (DI, 64)
                    m[q + 'dtp'] = _bf(sp['dt_proj'][:, rsl])  # (32, DIr)
                    m[q + 'dtb'] = _vec_tiles(sp['dt_bias'][rsl], JD)
                    m[q + 'A'] = np.ascontiguousarray(
                        (-np.exp(sp['A_log'][rsl])).astype(np.float32))
                    m[q + 'Dv'] = _vec_tiles(sp['D'][rsl], JD)
                    m[q + 'wo'] = _bf(sp['out_proj'][rsl, :])  # (DIr, D)
                m[pre + 'ga'] = _vec_tiles(ga, KD)

            sh, sc, ga = _mod3(cb, bp['mod_p_w'], bp['mod_p_b'])
            sh, sc, ga = sh[0], sc[0], ga[0]
            mp = bp['mlp']
            m[pre + 'w1'] = _bf(mp['w1'] * (1 + sc)[:, None])
            m[pre + 'b1'] = _vec_tiles(mp['w1'].T @ sh, HID // 128)
            m[pre + 'w2'] = _bf(mp['w2'] * (1 + sc)[:, None])
            m[pre + 'b2'] = _vec_tiles(mp['w2'].T @ sh, HID // 128)
            m[pre + 'w3'] = _bf(mp['w3'])
            m[pre + 'gp'] = _vec_tiles(ga, KD)

        sh, sc, _ = _mod3(cb, p['out_mod_w'], p['out_mod_b'])
        m['f_sc'] = _vec_tiles(1.0 + sc[0], KD)
        m['f_sh'] = _vec_tiles(sh[0], KD)
        m['wlm'] = wlm_r[r]  # (D, VS)

        in_maps.append(m)
    return in_maps


def build(nc):
    dram = {}

    def din(name, shape, dtype=F32):
        dram[name] = nc.dram_tensor(name, list(shape), dtype, kind="ExternalInput")
        return dram[name]

    din('ident', (128, 128), BF16)
    din('selbc', (2 * DS, 2 * DS * 128), BF16)
    din('x0T', (D, L))
    for i in range(N_LAYERS):
        pre = f'b{i}_'
        if i in ATTN:
            din(pre + 'wq', (D, NHr * HD), BF16)
            din(pre + 'bq', (1, 128, 1))
            din(pre + 'wk', (D, NHr * HD), BF16)
            din(pre + 'bk', (1, 128, 1))
            din(pre + 'wv', (D, NHr * HD), BF16)
            din(pre + 'bv', (1, 128, 1))
            din(pre + 'wo', (NHr * HD, D), BF16)
            din(pre + 'ga', (KD, 128, 1))
        else:
            for d in 'fb':
                q = pre + d + '_'
                din(q + 'wx', (D, DI), BF16)
                din(q + 'bx', (DI // 128, 128, 1))
                din(q + 'wz', (D, DIr), BF16)
                din(q + 'bz', (JD, 128, 1))
                din(q + 'xp', (DI, DT + 2 * DS), BF16)
                din(q + 'dtp', (DT, DIr), BF16)
                din(q + 'dtb', (JD, 128, 1))
                din(q + 'A', (DIr, DS))
                din(q + 'Dv', (JD, 128, 1))
                din(q + 'wo', (DIr, D), BF16)
            din(pre + 'ga', (KD, 128, 1))
        din(pre + 'w1', (D, HID), BF16)
        din(pre + 'b1', (HID // 128, 128, 1))
        din(pre + 'w2', (D, HID), BF16)
        din(pre + 'b2', (HID // 128, 128, 1))
        din(pre + 'w3', (HID, D), BF16)
        din(pre + 'gp', (KD, 128, 1))
    din('f_sc', (KD, 128, 1))
    din('f_sh', (KD, 128, 1))
    din('wlm', (D, VS), BF16)

    logits = nc.dram_tensor("logits", [L, VS], F32, kind="ExternalOutput")
    taps = None
    if DEBUG_TAPS:
        taps = nc.dram_tensor("taps", [N_LAYERS + 1, D, L], F32,
                              kind="ExternalOutput")

    groups = [[0, 1, 2, 3], [4, 5, 6, 7]]

    with tile.TileContext(nc) as tc:
        _Emitter(nc, tc, dram, logits, taps, groups).run()


class _Emitter:
    def __init__(self, nc, tc, dram, logits, taps, groups):
        self.nc, self.tc, self.dram = nc, tc, dram
        self.logits, self.taps, self.groups = logits, taps, groups

    def run(self):
        nc, tc, dram = self.nc, self.tc, self.dram
        with tc.tile_pool(name="stream", bufs=1) as p_stream, \
             tc.tile_pool(name="wts", bufs=2) as p_w, \
             tc.tile_pool(name="acts", bufs=2) as p_a, \
             tc.tile_pool(name="ssm", bufs=2) as p_s, \
             tc.tile_pool(name="small", bufs=2) as p_v, \
             tc.tile_pool(name="const", bufs=1) as p_c, \
             tc.tile_pool(name="psA", bufs=3, space="PSUM") as psA, \
             tc.tile_pool(name="psB", bufs=2, space="PSUM") as psB, \
             tc.tile_pool(name="psC", bufs=3, space="PSUM") as psC, \
             tc.tile_pool(name="dramp", bufs=1, space="DRAM") as p_d:
            self.p_stream, self.p_w, self.p_a, self.p_s = p_stream, p_w, p_a, p_s
            self.p_v, self.p_c = p_v, p_c
            self.psA, self.psB, self.psC, self.p_d = psA, psB, psC, p_d

            ones_col = p_c.tile([128, 1], BF16, name="ones_col")
            ones_row = p_c.tile([1, 128], BF16, name="ones_row")
            ident = p_c.tile([128, 128], BF16, name="identt")
            eps = p_c.tile([1, 1], F32, name="epsc")
            nc.vector.memset(eps[:], 1e-5)
            self.eps = eps
            onef = p_c.tile([128, 1], F32, name="onef")
            nc.vector.memset(onef[:], 1.0)
            self.onef = onef
            nc.vector.memset(ones_col[:], 1.0)
            nc.vector.memset(ones_row[:], 1.0)
            nc.sync.dma_start(ident[:], dram['ident'].ap())
            selbc = p_c.tile([2 * DS, 2 * DS * 128], BF16, name="selbct")
            nc.sync.dma_start(selbc[:], dram['selbc'].ap())
            self.selbc = selbc
            self.ones_col, self.ones_row, self.ident = ones_col, ones_row, ident

            xs = []
            for j in range(KD):
                xt = p_stream.tile([128, L], F32, name=f"xres{j}", tag=f"xres{j}")
                nc.sync.dma_start(xt[:], dram['x0T'].ap()[j * 128:(j + 1) * 128, :])
                xs.append(xt)
            self.xs = xs


            for i in range(N_BUILD_BLOCKS):
                pre = f'b{i}_'
                if i in ATTN:
                    self.attn_sublayer(pre)
                else:
                    self.mamba_sublayer(pre)
                self.mlp_sublayer(pre)
                if self.taps is not None:
                    for j in range(KD):
                        nc.sync.dma_start(
                            self.taps.ap()[i][j * 128:(j + 1) * 128, :], xs[j][:])

            self.head()

    # ---------- helpers ----------

    def layernorm(self, name):
        nc = self.nc
        xb, sq = [], []
        for j in range(KD):
            b = self.p_a.tile([128, L], BF16, name=f"{name}_xb{j}", tag="lnxb", bufs=3)
            s = self.p_a.tile([128, L], BF16, name=f"{name}_sq{j}", tag="lnsq", bufs=3)
            nc.vector.tensor_copy(b[:], self.xs[j][:])
            nc.scalar.activation(s[:], self.xs[j][:], AF.Square)
            xb.append(b)
            sq.append(s)
        s1 = self.psC.tile([1, L], F32, name=f"{name}_s1", tag="misc")
        s2 = self.psC.tile([1, L], F32, name=f"{name}_s2", tag="misc")
        for j in range(KD):
            nc.tensor.matmul(s1[:], self.ones_col[:], xb[j][:],
                             start=(j == 0), stop=(j == KD - 1))
        for j in range(KD):
            nc.tensor.matmul(s2[:], self.ones_col[:], sq[j][:],
                             start=(j == 0), stop=(j == KD - 1))
        m = self.p_v.tile([1, L], F32, name=f"{name}_m", tag="row_m", bufs=1)
        va = self.p_v.tile([1, L], F32, name=f"{name}_v", tag="row_v", bufs=1)
        nc.scalar.mul(m[:], s1[:], 1.0 / D)
        nc.scalar.mul(va[:], s2[:], 1.0 / D)
        msq = self.p_v.tile([1, L], F32, name=f"{name}_msq", tag="row_msq", bufs=1)
        nc.vector.tensor_mul(msq[:], m[:], m[:])
        nc.vector.tensor_sub(va[:], va[:], msq[:])
        sd = self.p_v.tile([1, L], F32, name=f"{name}_sd", tag="row_sd", bufs=1)
        nc.scalar.activation(sd[:], va[:], AF.Ln, bias=self.eps[:])
        rs = self.p_v.tile([1, L], F32, name=f"{name}_rs", tag="row_rs", bufs=1)
        nc.scalar.activation(rs[:], sd[:], AF.Exp, scale=-0.5)
        mr = self.p_v.tile([1, L], F32, name=f"{name}_mr", tag="row_mr", bufs=1)
        nc.vector.tensor_mul(mr[:], m[:], rs[:])
        rs_b = self.p_v.tile([1, L], BF16, name=f"{name}_rsb", tag="row_rsb", bufs=1)
        mr_b = self.p_v.tile([1, L], BF16, name=f"{name}_mrb", tag="row_mrb", bufs=1)
        nc.vector.tensor_copy(rs_b[:], rs[:])
        nc.vector.tensor_copy(mr_b[:], mr[:])
        R = self.psB.tile([128, L], F32, name=f"{name}_R", tag="bcast")
        MR = self.psB.tile([128, L], F32, name=f"{name}_MR", tag="bcast")
        nc.tensor.matmul(R[:], self.ones_row[:], rs_b[:], start=True, stop=True)
        nc.tensor.matmul(MR[:], self.ones_row[:], mr_b[:], start=True, stop=True)
        xln = []
        for j in range(KD):
            t1 = self.p_a.tile([128, L], F32, name=f"{name}_t1{j}", tag="lnt1", bufs=2)
            nc.vector.tensor_mul(t1[:], self.xs[j][:], R[:])
            o = self.p_a.tile([128, L], BF16, name=f"{name}_xln{j}", tag=f"xln{j}")
            nc.vector.tensor_sub(o[:], t1[:], MR[:])
            xln.append(o)
        return xln

    def matmul_T(self, name, xln, w_dram, fout, out_tag, bias_dram=None,
                 act=None, out_dtype=BF16, nkc=KD, out_bufs=2, psum_pool=None,
                 silu=False, softplus=False, wb_bufs=None, wtag=""):
        """out[fout, L] = W.T @ xln (+bias, act) in T-layout."""
        nc = self.nc
        mt = (fout + 127) // 128
        ktot = w_dram.shape[0]
        wband, kws = [], []
        for kc in range(nkc):
            kw = min(128, ktot - kc * 128)
            wb = self.p_w.tile([kw, fout], BF16, name=f"{name}_w{kc}",
                               tag=f"w{fout}_{kc}{wtag}", bufs=wb_bufs)
            nc.sync.dma_start(wb[:], w_dram.ap()[kc * 128:kc * 128 + kw, :])
            wband.append(wb)
            kws.append(kw)
        outs = []
        for mi in range(mt):
            mw = min(128, fout - mi * 128)
            pool = psum_pool or self.psA
            ps = pool.tile([128, L], F32, name=f"{name}_ps{mi}",
                           tag="acc" if pool is self.psA else "misc")
            for kc in range(nkc):
                nc.tensor.matmul(ps[:mw, :], wband[kc][:, mi * 128:mi * 128 + mw],
                                 xln[kc][:], start=(kc == 0), stop=(kc == nkc - 1))
            if isinstance(out_tag, list):
                otag, obufs = out_tag[mi]
            else:
                otag, obufs = f"{out_tag}{mi}", out_bufs
            o = self.p_a.tile([128, L], out_dtype, name=f"{name}_o{mi}",
                              tag=otag, bufs=obufs)
            bt = None
            if bias_dram is not None:
                bt = self.p_v.tile([128, 1], F32, name=f"{name}_b{mi}", tag="bias", bufs=4)
                nc.sync.dma_start(bt[:], bias_dram.ap()[mi])
            if silu:
                # o = u * sigmoid(u), u = ps + bias
                u = self.p_a.tile([128, L], F32, name=f"{name}_u{mi}",
                                  tag="silu_u", bufs=2)
                nc.scalar.activation(u[:mw, :], ps[:mw, :], AF.Identity,
                                     bias=bt[:mw, :] if bt is not None else 0.0)
                e = self.p_a.tile([128, L], F32, name=f"{name}_e{mi}",
                                  tag="silu_e", bufs=2)
                nc.scalar.activation(e[:mw, :], u[:mw, :], AF.Exp, scale=-1.0)
                nc.vector.tensor_scalar_add(e[:mw, :], e[:mw, :], 1.0)
                nc.vector.reciprocal(e[:mw, :], e[:mw, :])
                nc.vector.tensor_mul(o[:mw, :], u[:mw, :], e[:mw, :])
            elif softplus:
                # o = ln(1 + exp(ps + bias))
                e = self.p_a.tile([128, L], F32, name=f"{name}_e{mi}",
                                  tag="silu_e", bufs=2)
                nc.scalar.activation(e[:mw, :], ps[:mw, :], AF.Exp,
                                     bias=bt[:mw, :] if bt is not None else 0.0)
                nc.scalar.activation(o[:mw, :], e[:mw, :], AF.Ln,
                                     bias=self.onef[:mw, :])
            elif bt is not None:
                nc.scalar.activation(o[:mw, :], ps[:mw, :],
                                     act or AF.Identity, bias=bt[:mw, :])
            elif act is not None:
                nc.scalar.activation(o[:mw, :], ps[:mw, :], act)
            else:
                nc.scalar.copy(o[:mw, :], ps[:mw, :])
            outs.append(o)
        return outs

    def residual_direct(self, name, partials, gate_dram):
        nc = self.nc
        for j in range(KD):
            gt = self.p_v.tile([128, 1], F32, name=f"{name}_g{j}", tag="bias", bufs=4)
            nc.sync.dma_start(gt[:], gate_dram.ap()[j])
            nc.vector.scalar_tensor_tensor(
                self.xs[j][:], partials[j][:], gt[:], self.xs[j][:],
                ALU.mult, ALU.add)

    def allreduce_residual(self, name, partials, gate_dram):
        nc = self.nc
        ar_in = self.p_d.tile([D, L], BF16, name=f"{name}_arin", tag="arin",
                              bufs=2)
        ar_out = self.p_d.tile([D, L], BF16, name=f"{name}_arout", tag="arout",
                               bufs=2)
        for j in range(KD):
            nc.sync.dma_start(ar_in[j * 128:(j + 1) * 128, :], partials[j][:])
        nc.gpsimd.collective_compute(
            "AllReduce", ALU.add, replica_groups=self.groups,
            ins=[ar_in.opt()], outs=[ar_out.opt()])
        for j in range(KD):
            red = self.p_a.tile([128, L], BF16, name=f"{name}_red{j}", tag="red", bufs=2)
            nc.sync.dma_start(red[:], ar_out[j * 128:(j + 1) * 128, :])
            gt = self.p_v.tile([128, 1], F32, name=f"{name}_g{j}", tag="bias", bufs=4)
            nc.sync.dma_start(gt[:], gate_dram.ap()[j])
            nc.vector.scalar_tensor_tensor(
                self.xs[j][:], red[:], gt[:], self.xs[j][:], ALU.mult, ALU.add)

    # ---------- sublayers ----------

    def ssm(self, q, xln, rev):
        """One SSM instance; returns JD gated bf16 tiles [128, L] (in the
        instance's own time direction)."""
        nc, dram = self.nc, self.dram
        sfx = 'b' if rev else 'f'
        xf_tags = [(f"xfs{m}", 2) if m < JD else ("xfr", 4)
                   for m in range(DI // 128)]
        x_full = self.matmul_T(q + 'x', xln, dram[q + 'wx'], DI, out_tag=xf_tags,
                               bias_dram=dram[q + 'bx'])
        zs = self.matmul_T(q + 'z', xln, dram[q + 'wz'], DIr, out_tag="zs",
                           bias_dram=dram[q + 'bz'], silu=True)
        dbc = self.matmul_T(q + 'dbc', x_full, dram[q + 'xp'], DT + 2 * DS,
                            out_tag="dbc", out_dtype=F32, nkc=DI // 128,
                            out_bufs=1, psum_pool=self.psC)[0]
        dt_bf = self.p_a.tile([DT, L], BF16, name=q + "dtbf", tag="dtbf")
        nc.vector.tensor_copy(dt_bf[:], dbc[:DT, :])
        bc_bf = self.p_a.tile([2 * DS, L], BF16, name=q + "bcbf", tag="bcbf")
        nc.vector.tensor_copy(bc_bf[:], dbc[DT:DT + 2 * DS, :])

        delta = self.matmul_T(q + 'dl', [dt_bf], dram[q + 'dtp'], DIr,
                              out_tag="dl", bias_dram=dram[q + 'dtb'],
                              softplus=True, out_dtype=F32, nkc=1,
                              out_bufs=1)
        At = self.p_v.tile([128, JD, DS], F32, name=q + "A", tag="Acol")
        nc.sync.dma_start(At[:], dram[q + 'A'].ap().rearrange(
            "(j p) s -> p j s", p=128))

        dx = []
        for j in range(JD):
            o = self.p_s.tile([128, L], BF16, name=f"{q}dx{j}",
                              tag=f"dx{sfx}{j}", bufs=1)
            nc.vector.tensor_mul(o[:], delta[j][:], x_full[j][:])
            dx.append(o)

        yacc = [self.p_s.tile([128, L], F32, name=f"{q}y{j}",
                              tag=f"y{sfx}{j}", bufs=1) for j in range(JD)]

        for s in range(DS):
            Bb = self.psB.tile([128, L], F32, name=f"{q}Bb{s}", tag="bcast")
            Cb = self.psB.tile([128, L], F32, name=f"{q}Cb{s}", tag="bcast")
            nc.tensor.matmul(Bb[:], self.selbc[:, s * 128:(s + 1) * 128],
                             bc_bf[:], start=True, stop=True)
            nc.tensor.matmul(Cb[:], self.selbc[:, (DS + s) * 128:(DS + s + 1) * 128],
                             bc_bf[:], start=True, stop=True)
            Bbs = self.p_s.tile([128, L], BF16, name=f"{q}Bbs{s}", tag="Bbs")
            Cbs = self.p_s.tile([128, L], BF16, name=f"{q}Cbs{s}", tag="Cbs")
            nc.scalar.copy(Bbs[:], Bb[:])
            nc.scalar.copy(Cbs[:], Cb[:])
            for j in range(JD):
                dA = self.p_s.tile([128, L], BF16, name=f"{q}dA{s}_{j}", tag="dA")
                nc.scalar.activation(dA[:], delta[j][:], AF.Exp,
                                     scale=At[:, j, s:s + 1])
                dbx = self.p_s.tile([128, L], BF16, name=f"{q}dbx{s}_{j}",
                                    tag="dbx")
                nc.gpsimd.tensor_mul(dbx[:], dx[j][:], Bbs[:])
                h = self.p_s.tile([128, L], BF16, name=f"{q}h{s}_{j}", tag="h")
                nc.vector.tensor_tensor_scan(h[:], dA[:], dbx[:], 0.0,
                                             ALU.mult, ALU.add)
                hc = self.p_s.tile([128, L], BF16, name=f"{q}hc{s}_{j}", tag="hc")
                nc.gpsimd.tensor_mul(hc[:], h[:], Cbs[:])
                if s == 0:
                    nc.vector.tensor_copy(yacc[j][:], hc[:])
                else:
                    nc.vector.tensor_add(yacc[j][:], yacc[j][:], hc[:])

        gated = []
        for j in range(JD):
            Dt = self.p_v.tile([128, 1], F32, name=f"{q}D{j}", tag="bias", bufs=4)
            nc.sync.dma_start(Dt[:], dram[q + 'Dv'].ap()[j])
            nc.vector.scalar_tensor_tensor(
                yacc[j][:], x_full[j][:], Dt[:], yacc[j][:], ALU.mult, ALU.add)
            gt = self.p_s.tile([128, L], BF16, name=f"{q}gt{j}",
                               tag=f"gt{sfx}{j}", bufs=1)
            nc.vector.tensor_mul(gt[:], yacc[j][:], zs[j][:])
            gated.append(gt)
        return gated

    def mamba_sublayer(self, pre):
        nc, dram = self.nc, self.dram
        xln = self.layernorm(pre + "ln1")
        # reversed stream for the backward ssm
        xln_r = []
        for j in range(KD):
            o = self.p_a.tile([128, L], BF16, name=f"{pre}xlr{j}", tag=f"xlr{j}")
            nc.gpsimd.tensor_copy(o[:], xln[j][:, ::-1])
            xln_r.append(o)
        gated_f = self.ssm(pre + 'f_', xln, rev=False)
        gated_b = self.ssm(pre + 'b_', xln_r, rev=True)
        # un-reverse the backward gated output
        gated_br = []
        for j in range(JD):
            o = self.p_s.tile([128, L], BF16, name=f"{pre}gbr{j}", tag=f"gbr{j}",
                              bufs=1)
            nc.gpsimd.tensor_copy(o[:], gated_b[j][:, ::-1])
            gated_br.append(o)

        wf = self.p_w.tile([128, JD, D], BF16, name=pre + "wof", tag="wof", bufs=1)
        wb = self.p_w.tile([128, JD, D], BF16, name=pre + "wob", tag="wob", bufs=1)
        nc.sync.dma_start(wf[:], dram[pre + 'f_wo'].ap().rearrange(
            "(j p) d -> p j d", p=128))
        nc.sync.dma_start(wb[:], dram[pre + 'b_wo'].ap().rearrange(
            "(j p) d -> p j d", p=128))
        partials = []
        for mi in range(KD):
            ps = self.psA.tile([128, L], F32, name=f"{pre}op{mi}", tag="acc")
            for j in range(JD):
                nc.tensor.matmul(ps[:], wf[:, j, mi * 128:(mi + 1) * 128],
                                 gated_f[j][:], start=(j == 0), stop=False)
            for j in range(JD):
                nc.tensor.matmul(ps[:], wb[:, j, mi * 128:(mi + 1) * 128],
                                 gated_br[j][:], start=False, stop=(j == JD - 1))
            o = self.p_a.tile([128, L], BF16, name=f"{pre}par{mi}",
                              tag=f"par{mi}", bufs=1)
            nc.scalar.copy(o[:], ps[:])
            partials.append(o)
        self.allreduce_residual(pre + "m", partials, dram[pre + 'ga'])

    def attn_sublayer(self, pre):
        nc, dram = self.nc, self.dram
        xln = self.layernorm(pre + "ln1")
        q_sb = self.matmul_T(pre + 'q', xln, dram[pre + 'wq'], NHr * HD,
                             out_tag="qsb", bias_dram=dram[pre + 'bq'])[0]
        k_sb = self.matmul_T(pre + 'k', xln, dram[pre + 'wk'], NHr * HD,
                             out_tag="ksb", bias_dram=dram[pre + 'bk'])[0]
        wv = self.p_w.tile([128, KD, NHr * HD], BF16, name=pre + "wv", tag="wv")
        nc.sync.dma_start(wv[:], dram[pre + 'wv'].ap().rearrange(
            "(c p) h -> p c h", p=128))
        v_sb = []
        for tm in range(KD):
            ps = self.psC.tile([128, NHr * HD], F32, name=f"{pre}vps{tm}",
                               tag="misc")
            for kc in range(KD):
                nc.tensor.matmul(ps[:], xln[kc][:, tm * 128:(tm + 1) * 128],
                                 wv[:, kc, :], start=(kc == 0),
                                 stop=(kc == KD - 1))
            vt = self.p_a.tile([128, NHr * HD], BF16, name=f"{pre}v{tm}",
                               tag=f"vsb{tm}")
            nc.scalar.copy(vt[:], ps[:])
            v_sb.append(vt)
        bv = self.p_v.tile([128, 1], F32, name=pre + "bv", tag="bvb", bufs=2)
        nc.sync.dma_start(bv[:], dram[pre + 'bv'].ap()[0])

        oT = self.p_a.tile([128, L], BF16, name=pre + "oT", tag="oT")
        for h in range(NHr):
            hof = h * HD
            pT = self.p_a.tile([128, KD, L], BF16, name=f"{pre}pT{h}", tag="pT",
                               bufs=1)
            for tq in range(KD):
                sc_ps = self.psA.tile([128, L], F32, name=f"{pre}s{h}_{tq}",
                                      tag="acc")
                nc.tensor.matmul(sc_ps[:],
                                 q_sb[hof:hof + HD, tq * 128:(tq + 1) * 128],
                                 k_sb[hof:hof + HD, :], start=True, stop=True)
                nmax = self.p_v.tile([128, 1], F32, name=f"{pre}mx{h}{tq}",
                                     tag="bias", bufs=4)
                nc.vector.tensor_reduce(nmax[:], sc_ps[:], AX.X, ALU.max,
                                        negate=True)
                pr = self.p_a.tile([128, L], BF16, name=f"{pre}pr{h}{tq}",
                                   tag="probs")
                ssum = self.p_v.tile([128, 1], F32, name=f"{pre}sm{h}{tq}",
                                     tag="bias", bufs=4)
                nc.scalar.activation(pr[:], sc_ps[:], AF.Exp, bias=nmax[:],
                                     accum_out=ssum[:])
                rcp = self.p_v.tile([128, 1], F32, name=f"{pre}rs{h}{tq}",
                                    tag="bias", bufs=4)
                nc.vector.reciprocal(rcp[:], ssum[:])
                nc.vector.tensor_scalar_mul(pr[:], pr[:], rcp[:])
                for tk in range(KD):
                    tp = self.psC.tile([128, 128], BF16, name=f"{pre}tp{h}{tq}{tk}",
                                       tag="misc")
                    nc.tensor.transpose(tp[:], pr[:, tk * 128:(tk + 1) * 128],
                                        self.ident[:])
                    nc.scalar.copy(pT[:, tk, tq * 128:(tq + 1) * 128], tp[:])
            ops = self.psC.tile([HD, L], F32, name=f"{pre}ops{h}", tag="misc")
            for tk in range(KD):
                nc.tensor.matmul(ops[:], v_sb[tk][:, hof:hof + HD], pT[:, tk, :],
                                 start=(tk == 0), stop=(tk == KD - 1))
            nc.scalar.activation(oT[hof:hof + HD, :], ops[:], AF.Identity,
                                 bias=bv[hof:hof + HD, :])
        partials = self.matmul_T(pre + 'o', [oT], dram[pre + 'wo'], D,
                                 out_tag="par", out_dtype=BF16, nkc=1,
                                 out_bufs=1)
        self.allreduce_residual(pre + "a", partials, dram[pre + 'ga'])

    def mlp_sublayer(self, pre):
        nc, dram = self.nc, self.dram
        xln = self.layernorm(pre + "ln2")
        nmt = HID // 128
        wb1, wb2 = [], []
        for kc in range(KD):
            w1t = self.p_w.tile([128, HID], BF16, name=f"{pre}w1_{kc}",
                                tag=f"w{HID}_{kc}")
            w2t = self.p_w.tile([128, HID], BF16, name=f"{pre}w2_{kc}",
                                tag=f"w{HID}_{kc}")
            nc.sync.dma_start(w1t[:], dram[pre + 'w1'].ap()[kc * 128:(kc + 1) * 128, :])
            nc.sync.dma_start(w2t[:], dram[pre + 'w2'].ap()[kc * 128:(kc + 1) * 128, :])
            wb1.append(w1t)
            wb2.append(w2t)
        g = []
        for mi in range(nmt):
            ms = slice(mi * 128, (mi + 1) * 128)
            ps1 = self.psA.tile([128, L], F32, name=f"{pre}ps1_{mi}", tag="acc")
            for kc in range(KD):
                nc.tensor.matmul(ps1[:], wb1[kc][:, ms], xln[kc][:],
                                 start=(kc == 0), stop=(kc == KD - 1))
            b1 = self.p_v.tile([128, 1], F32, name=f"{pre}b1_{mi}", tag="bias",
                               bufs=4)
            nc.sync.dma_start(b1[:], dram[pre + 'b1'].ap()[mi])
            u = self.p_a.tile([128, L], F32, name=f"{pre}u{mi}", tag="silu_u",
                              bufs=2)
            nc.scalar.activation(u[:], ps1[:], AF.Identity, bias=b1[:])
            e = self.p_a.tile([128, L], F32, name=f"{pre}e{mi}", tag="silu_e",
                              bufs=2)
            nc.scalar.activation(e[:], u[:], AF.Exp, scale=-1.0)
            nc.vector.tensor_scalar_add(e[:], e[:], 1.0)
            nc.vector.reciprocal(e[:], e[:])
            m1 = self.p_a.tile([128, L], BF16, name=f"{pre}m1_{mi}", tag="m1r",
                               bufs=2)
            nc.vector.tensor_mul(m1[:], u[:], e[:])
            ps2 = self.psA.tile([128, L], F32, name=f"{pre}ps2_{mi}", tag="acc")
            for kc in range(KD):
                nc.tensor.matmul(ps2[:], wb2[kc][:, ms], xln[kc][:],
                                 start=(kc == 0), stop=(kc == KD - 1))
            b2 = self.p_v.tile([128, 1], F32, name=f"{pre}b2_{mi}", tag="bias",
                               bufs=4)
            nc.sync.dma_start(b2[:], dram[pre + 'b2'].ap()[mi])
            m2 = self.p_a.tile([128, L], BF16, name=f"{pre}m2_{mi}", tag="m2r",
                               bufs=2)
            nc.scalar.activation(m2[:], ps2[:], AF.Identity, bias=b2[:])
            gt = self.p_a.tile([128, L], BF16, name=f"{pre}g{mi}",
                               tag=f"gf{mi}", bufs=1)
            nc.vector.tensor_mul(gt[:], m1[:], m2[:])
            g.append(gt)
        wb3 = self.p_w.tile([128, nmt, D], BF16, name=pre + "w3t", tag="w3big",
                            bufs=1)
        nc.sync.dma_start(wb3[:], dram[pre + 'w3'].ap().rearrange(
            "(c p) d -> p c d", p=128))
        partials = []
        for m in range(KD):
            pso = self.psC.tile([128, L], F32, name=f"{pre}pso{m}", tag="misc")
            for kc in range(nmt):
                nc.tensor.matmul(pso[:], wb3[:, kc, m * 128:(m + 1) * 128],
                                 g[kc][:], start=(kc == 0), stop=(kc == nmt - 1))
            o = self.p_a.tile([128, L], BF16, name=f"{pre}mpar{m}",
                              tag=f"par{m}", bufs=1)
            nc.scalar.copy(o[:], pso[:])
            partials.append(o)
        self.residual_direct(pre + "p", partials, dram[pre + 'gp'])

    def head(self):
        nc, dram = self.nc, self.dram
        xln = self.layernorm("fln")
        hf = []
        for j in range(KD):
            sc = self.p_v.tile([128, 1], F32, name=f"fsc{j}", tag="bias", bufs=4)
            sh = self.p_v.tile([128, 1], F32, name=f"fsh{j}", tag="bias", bufs=4)
            nc.sync.dma_start(sc[:], dram['f_sc'].ap()[j])
            nc.sync.dma_start(sh[:], dram['f_sh'].ap()[j])
            o = self.p_a.tile([128, L], BF16, name=f"hf{j}", tag=f"hf{j}", bufs=1)
            nc.vector.tensor_scalar(o[:], xln[j][:], sc[:], sh[:], ALU.mult,
                                    ALU.add)
            hf.append(o)
        if self.taps is not None:
            for j in range(KD):
                tap32 = self.p_a.tile([128, L], F32, name=f"tapc{j}", tag="red", bufs=2)
                nc.vector.tensor_copy(tap32[:], hf[j][:])
                nc.sync.dma_start(
                    self.taps.ap()[N_LAYERS][j * 128:(j + 1) * 128, :], tap32[:])
        for vc in range(NVC):
            vw = min(512, VS - vc * 512)
            wt = self.p_w.tile([128, KD, 512], BF16, name=f"wlm{vc}", tag="wlm")
            nc.sync.dma_start(wt[:, :, :vw], dram['wlm'].ap().rearrange(
                "(c p) v -> p c v", p=128)[:, :, vc * 512:vc * 512 + vw])
            for tm in range(KD):
                ps = self.psA.tile([128, 512], F32, name=f"lg{vc}_{tm}", tag="acc")
                for kc in range(KD):
                    nc.tensor.matmul(ps[:, :vw],
                                     hf[kc][:, tm * 128:(tm + 1) * 128],
                                     wt[:, kc, :vw], start=(kc == 0),
                                     stop=(kc == KD - 1))
                o = self.p_a.tile([128, 512], F32, name=f"lo{vc}_{tm}", tag="lo", bufs=2)
                nc.scalar.copy(o[:, :vw], ps[:, :vw])
                nc.sync.dma_start(
                    self.logits.ap()[tm * 128:(tm + 1) * 128,
                                     vc * 512:vc * 512 + vw], o[:, :vw])


def get_compiled():
    if "nc" not in _cache:
        nc = bacc.Bacc("TRN2", target_bir_lowering=False, debug=False,
                       num_devices=8)
        build(nc)
        nc.compile()
        _cache["nc"] = nc
    return _cache["nc"]


def kernel(input_ids, t, params):
    nc = get_compiled()
    in_maps = prepare_inputs(input_ids, t, params)
    trace = bool(int(os.environ.get("KERNEL_TRACE", "0")))
    res = run_bass_kernel_spmd(nc, in_maps, list(range(8)), trace=trace)
    _cache["last_results"] = res
    out = np.empty((2, L, V), np.float32)
    for g in range(2):
        for r in range(TP):
            out[g, :, r * VS:(r + 1) * VS] = res.results[g * TP + r]["logits"]
    return out
